# revision 17
# baseline (speedup 1.0000x reference)
"""CTC loss kernel for Trainium2 (Bass/Tile), data-parallel over batch on 8 NeuronCores.

Problem: input [T=256, B=32, C=6625] f32 logits, label [B=32, L=25] int.
Reference: log_softmax over C, gather extended-label log-probs, CTC forward DP
over T steps, mean loss / T.

Strategy (per core, B_local = 4 sequences):
  - The memory-bound part is reading the 27.1 MB input shard. Tiles of
    [128 partitions = 4 b-blocks x 32 t, C] are DMA'd in; the ACT engine
    computes exp(x) with an accumulated row sum (=> softmax denominator
    without any max-pass or log), GPSIMD ap_gather pulls the 51
    extended-label logits per row, ACT exponentiates those with a +lnK bias,
    and DVE scales by the reciprocal row-sum => p'_t[b,s] = K*softmax values.
  - The CTC forward DP runs in probability space (scaled forward algorithm)
    entirely on DVE with 4 fused scalar_tensor_tensor ops per time step on
    [4, 51] tiles, renormalizing every 8 steps by 1/c (c = per-step alpha
    sums accumulated for free by the STT accum_out).
  - Outputs per core: final alpha state [4, 53] and the c table [4, 256].
    The host reconstructs the loss in f64 (logs, end-state sums, masking)
    and averages across cores. Host-side math touches only O(B*T) scalars.
"""

import numpy as np

import concourse.bacc as bacc
import concourse.bass as bass
import concourse.tile as tile
from concourse import mybir
from concourse.bass_utils import run_bass_kernel_spmd

T, B, C = 256, 32, 6625
L = 25
S = 2 * L + 1  # 51
NCORES = 8
BLOC = B // NCORES  # 4
TQ = 32  # t-steps per tile
NT = T // TQ  # 8 tiles
NIDX = 64  # gather width (51 used, padded to 64)
PQW = 2 * NIDX  # combined p|q column block per t
RENORM = 8
LNK = float(np.log(np.float32(C)))

_CACHE = {}
LAST_RESULT = None  # BassKernelResults of the most recent run (for profiling)


def _build_bass(p_dt, state_dt):
    nc = bacc.Bacc("TRN2", debug=False, num_devices=NCORES)
    f32 = mybir.dt.float32
    x_in = nc.declare_dram_parameter("x", [T, BLOC, C], f32, isOutput=False)
    idx_in = nc.declare_dram_parameter("idx", [128, NIDX // 16], mybir.dt.int16, isOutput=False)
    msk_in = nc.declare_dram_parameter("msk", [128, NIDX], f32, isOutput=False)
    a_out = nc.declare_dram_parameter("a_out", [BLOC, S + 2], f32, isOutput=True)
    ct_out = nc.declare_dram_parameter("ct_out", [BLOC, T], f32, isOutput=True)

    with tile.TileContext(nc) as tc:
        with (
            tc.tile_pool(name="xp", bufs=2) as xp,
            tc.tile_pool(name="scr", bufs=1) as scr,
            tc.tile_pool(name="sm", bufs=3) as sm,
            tc.tile_pool(name="st", bufs=1) as st,
        ):
            # persistent state
            idx_t = st.tile([128, NIDX // 16], mybir.dt.int16)
            msk_t = st.tile([128, NIDX], p_dt)
            # one pq tensor per t-quarter so tile i+1's assemble has no WAR
            # against the DP chunk reading tile i (whole-tensor dep tracking)
            pq_tiles = [
                st.tile([BLOC, TQ * PQW], p_dt, name=f"pq{i}") for i in range(NT)
            ]
            # W packs both DP state vectors: A (alpha) at cols 0..52 (pads 0,1),
            # B (m[j]*alpha[j-2]) at cols 64..117 (pads 64,65). The A/B update
            # is then ONE double-width STT with a [(64,2),(1,51)] output AP.
            W = st.tile([BLOC, 2 * NIDX], state_dt)
            u = st.tile([BLOC, S], state_dt)
            z = st.tile([BLOC, S], state_dt)
            ct = st.tile([BLOC, T], f32)
            r = st.tile([BLOC, 1], f32)

            lnk_t = st.tile([128, 1], f32)
            nc.vector.memset(lnk_t, LNK)
            nc.sync.dma_start(out=idx_t, in_=idx_in[:])
            nc.gpsimd.dma_start(out=msk_t, in_=msk_in[:])  # SWDGE casts f32->p_dt
            nc.vector.memset(W, 0.0)
            nc.vector.memset(ct, 1.0)

            x_bmaj = x_in[:].rearrange("t b c -> b t c")  # [BLOC, T, C]

            for i in range(NT):
                t0 = i * TQ
                xt = xp.tile([128, C], f32, tag="xt")
                # partitions: p = 32*b + tau  <-  x[t0+tau, b, :]
                # One DMA per b so each transfer's outer dim is 32 (not 4):
                # descriptors then spread across the SDMA engine pool instead
                # of landing on 4 engines.
                for b in range(BLOC):
                    nc.gpsimd.dma_start(
                        out=xt[32 * b : 32 * (b + 1), :],
                        in_=x_bmaj[b, t0 : t0 + TQ, :],
                    )
                sums = sm.tile([128, 1], f32, tag="sums")
                scratch = scr.tile([128, C], p_dt, tag="scratch")
                nc.scalar.activation(
                    out=scratch,
                    in_=xt,
                    func=mybir.ActivationFunctionType.Exp,
                    accum_out=sums,
                )
                rec = sm.tile([128, 1], f32, tag="rec")
                nc.vector.reciprocal(out=rec, in_=sums)
                graw = sm.tile([128, NIDX], f32, tag="graw")
                nc.gpsimd.ap_gather(
                    out_ap=graw[:],
                    in_ap=xt[:],
                    idxs_ap=idx_t[:],
                    channels=128,
                    num_elems=C,
                    d=1,
                    num_idxs=NIDX,
                )
                e = sm.tile([128, NIDX], f32, tag="e")
                nc.scalar.activation(
                    out=e,
                    in_=graw,
                    func=mybir.ActivationFunctionType.Exp,
                    bias=lnk_t[:],
                )
                pqs = sm.tile([128, PQW], p_dt, tag="pqs")
                nc.vector.tensor_scalar_mul(
                    out=pqs[:, 0:NIDX], in0=e, scalar1=rec
                )
                nc.vector.tensor_mul(
                    out=pqs[:, NIDX:PQW], in0=pqs[:, 0:NIDX], in1=msk_t
                )
                # assemble: pq_i[b, tau*128 + j] = pqs[32*b + tau, j]
                pq = pq_tiles[i]
                for b in range(BLOC):
                    nc.sync.dma_start(
                        out=pq[b : b + 1, :],
                        in_=pqs[32 * b : 32 * (b + 1), :],
                    )

                # ---- DP chunk for t in [t0, t0+TQ) ----
                for t in range(max(t0, 1), t0 + TQ):
                    if t == 1:
                        # init alpha_0 from p'_0 at s=0,1
                        nc.vector.tensor_copy(out=W[:, 2:4], in_=pq[:, 0:2])
                    # u[s] = alpha[s] + alpha[s-1]
                    nc.vector.scalar_tensor_tensor(
                        out=u,
                        in0=W[:, 2 : 2 + S],
                        scalar=0.0,
                        in1=W[:, 1 : 1 + S],
                        op0=mybir.AluOpType.bypass,
                        op1=mybir.AluOpType.add,
                    )
                    # z[s] = u[s] + B[s]  (B[s] = m[s]*alpha[s-2])
                    nc.vector.scalar_tensor_tensor(
                        out=z,
                        in0=u,
                        scalar=0.0,
                        in1=W[:, NIDX : NIDX + S],
                        op0=mybir.AluOpType.bypass,
                        op1=mybir.AluOpType.add,
                    )
                    if t % RENORM == 0:
                        nc.vector.reciprocal(out=r, in_=ct[:, t - 1 : t])
                        op0, sc = mybir.AluOpType.mult, r[:]
                    else:
                        op0, sc = mybir.AluOpType.bypass, 0.0
                    # fused update: {A[2:53], B[66:117]} = (z*r) * {p_t, q_t}
                    z_b = bass.AP(
                        tensor=z.tensor, offset=z.offset, ap=[z.ap[0], [0, 2], [1, S]]
                    )
                    w_o = bass.AP(
                        tensor=W.tensor,
                        offset=W.offset + 2,
                        ap=[W.ap[0], [NIDX, 2], [1, S]],
                    )
                    pq_t = bass.AP(
                        tensor=pq.tensor,
                        offset=pq.offset + (t - t0) * PQW,
                        ap=[pq.ap[0], [NIDX, 2], [1, S]],
                    )
                    nc.vector.scalar_tensor_tensor(
                        out=w_o,
                        in0=z_b,
                        scalar=sc,
                        in1=pq_t,
                        op0=op0,
                        op1=mybir.AluOpType.mult,
                        accum_out=ct[:, t : t + 1],
                    )

            nc.sync.dma_start(out=a_out[:], in_=W[:, 0 : S + 2])
            nc.sync.dma_start(out=ct_out[:], in_=ct)
    nc.finalize()
    return nc


def _prep_labels(label):
    pos = np.arange(L)
    key = np.where(label != 0, pos, L + pos)
    order = np.argsort(key, axis=1, kind="stable")
    packed = np.take_along_axis(label, order, axis=1)
    label_len = (label != 0).sum(1)
    ext = np.zeros((B, S), dtype=np.int64)
    ext[:, 1::2] = packed
    ext_m2 = np.pad(ext, ((0, 0), (2, 0)))[:, :S]
    skip = (np.arange(S) >= 2) & (ext != 0) & (ext != ext_m2)
    return ext, skip.astype(np.float32), label_len


def _core_inputs(x, ext, skip, core):
    bs = core * BLOC
    xc = np.ascontiguousarray(x[:, bs : bs + BLOC, :], dtype=np.float32)
    # gather index tile: 16-partition group g covers b = g//2; index j of the
    # group's list lives at [16*g + j%16, j//16]
    idx = np.zeros((128, NIDX // 16), dtype=np.int16)
    for g in range(8):
        b = bs + g // 2
        lst = np.zeros(NIDX, dtype=np.int16)
        lst[:S] = ext[b].astype(np.int16)
        idx[16 * g : 16 * (g + 1), :] = lst.reshape(NIDX // 16, 16).T
    # mask tile (pre-shifted): msk[32*b + tau, c] = skip[b, c+2]
    msk = np.zeros((128, NIDX), dtype=np.float32)
    for bl in range(BLOC):
        row = np.zeros(NIDX, dtype=np.float32)
        row[: S - 2] = skip[bs + bl, 2:]
        msk[32 * bl : 32 * (bl + 1), :] = row[None, :]
    return {"x": xc, "idx": idx, "msk": msk}


def _host_loss(a_fin, ct, label_len, core):
    """a_fin [BLOC, S+2] f32, ct [BLOC, T] f32 -> per-b losses (f64)."""
    losses = np.zeros(BLOC)
    renorm_ts = np.arange(RENORM, T, RENORM)
    for bl in range(BLOC):
        l = int(label_len[core * BLOC + bl])
        i0, i1 = max(2 * l - 1, 0), 2 * l
        end = float(a_fin[bl, 2 + i0]) + float(a_fin[bl, 2 + i1])
        cts = ct[bl, renorm_ts - 1].astype(np.float64)
        if end <= 0 or not np.isfinite(end) or np.any(~np.isfinite(cts)) or np.any(cts <= 0):
            losses[bl] = 0.0
            continue
        logP = np.log(end) + np.log(cts).sum() - T * LNK
        lb = -logP
        losses[bl] = 0.0 if lb > 1e8 else lb
    return losses


def kernel(input, label, p_dtype="bfloat16", state_dtype="float32", trace=False):
    global LAST_RESULT
    x = np.asarray(input, dtype=np.float32)
    label = np.asarray(label).astype(np.int64)
    assert x.shape == (T, B, C) and label.shape == (B, L)
    ext, skip, label_len = _prep_labels(label)

    key = (p_dtype, state_dtype)
    if key not in _CACHE:
        _CACHE[key] = _build_bass(
            getattr(mybir.dt, p_dtype), getattr(mybir.dt, state_dtype)
        )
    nc = _CACHE[key]

    in_maps = [_core_inputs(x, ext, skip, c) for c in range(NCORES)]
    res = run_bass_kernel_spmd(nc, in_maps, list(range(NCORES)), trace=trace)
    LAST_RESULT = res
    losses = np.concatenate(
        [
            _host_loss(
                np.asarray(res.results[c]["a_out"], np.float32),
                np.asarray(res.results[c]["ct_out"], np.float32),
                label_len,
                c,
            )
            for c in range(NCORES)
        ]
    )
    return np.float32(losses.mean() / T)


# revision 18
# speedup vs baseline: 1.0729x; 1.0729x over previous
"""CTC loss kernel for Trainium2 (Bass/Tile), data-parallel over batch on 8 NeuronCores.

Problem: input [T=256, B=32, C=6625] f32 logits, label [B=32, L=25] int.
Reference: log_softmax over C, gather extended-label log-probs, CTC forward DP
over T steps, mean loss / T.

Strategy (per core, B_local = 4 sequences):
  - The memory-bound part is reading the 27.1 MB input shard. Tiles of
    [128 partitions = 4 b-blocks x 32 t, C] are DMA'd in; the ACT engine
    computes exp(x) with an accumulated row sum (=> softmax denominator
    without any max-pass or log), GPSIMD ap_gather pulls the 51
    extended-label logits per row, ACT exponentiates those with a +lnK bias,
    and DVE scales by the reciprocal row-sum => p'_t[b,s] = K*softmax values.
  - The CTC forward DP runs in probability space (scaled forward algorithm)
    entirely on DVE with 4 fused scalar_tensor_tensor ops per time step on
    [4, 51] tiles, renormalizing every 8 steps by 1/c (c = per-step alpha
    sums accumulated for free by the STT accum_out).
  - Outputs per core: final alpha state [4, 53] and the c table [4, 256].
    The host reconstructs the loss in f64 (logs, end-state sums, masking)
    and averages across cores. Host-side math touches only O(B*T) scalars.
"""

import numpy as np

import concourse.bacc as bacc
import concourse.bass as bass
import concourse.tile as tile
from concourse import mybir
from concourse.bass_utils import run_bass_kernel_spmd

T, B, C = 256, 32, 6625
L = 25
S = 2 * L + 1  # 51
NCORES = 8
BLOC = B // NCORES  # 4
TQ = 32  # t-steps per tile
NT = T // TQ  # 8 tiles
NIDX = 64  # gather width (51 used, padded to 64)
PQW = 2 * NIDX  # combined p|q column block per t
RENORM = 8
LNK = float(np.log(np.float32(C)))

_CACHE = {}
LAST_RESULT = None  # BassKernelResults of the most recent run (for profiling)


def _build_bass(p_dt, state_dt):
    nc = bacc.Bacc("TRN2", debug=False, num_devices=NCORES)
    f32 = mybir.dt.float32
    x_in = nc.declare_dram_parameter("x", [T, BLOC, C], f32, isOutput=False)
    idx_in = nc.declare_dram_parameter("idx", [128, NIDX // 16], mybir.dt.int16, isOutput=False)
    msk_in = nc.declare_dram_parameter("msk", [128, NIDX], f32, isOutput=False)
    a_out = nc.declare_dram_parameter("a_out", [BLOC, S + 2], f32, isOutput=True)
    ct_out = nc.declare_dram_parameter("ct_out", [BLOC, T], f32, isOutput=True)

    with tile.TileContext(nc) as tc:
        with (
            tc.tile_pool(name="xp", bufs=2) as xp,
            tc.tile_pool(name="scr", bufs=1) as scr,
            tc.tile_pool(name="sm", bufs=3) as sm,
            tc.tile_pool(name="st", bufs=1) as st,
        ):
            # persistent state
            idx_t = st.tile([128, NIDX // 16], mybir.dt.int16)
            msk_t = st.tile([128, NIDX], p_dt)
            # one pq tensor per t-quarter so tile i+1's assemble has no WAR
            # against the DP chunk reading tile i (whole-tensor dep tracking)
            pq_tiles = [
                st.tile([BLOC, TQ * PQW], p_dt, name=f"pq{i}") for i in range(NT)
            ]
            # W packs both DP state vectors: A (alpha) at cols 0..52 (pads 0,1),
            # B (m[j]*alpha[j-2]) at cols 64..117 (pads 64,65). The A/B update
            # is then ONE double-width STT with a [(64,2),(1,51)] output AP.
            W = st.tile([BLOC, 2 * NIDX], state_dt)
            u = st.tile([BLOC, S], state_dt)
            z = st.tile([BLOC, S], state_dt)
            ct = st.tile([BLOC, T], f32)
            r = st.tile([BLOC, 1], f32)

            lnk_t = st.tile([128, 1], f32)
            nc.vector.memset(lnk_t, LNK)
            nc.sync.dma_start(out=idx_t, in_=idx_in[:])
            nc.gpsimd.dma_start(out=msk_t, in_=msk_in[:])  # SWDGE casts f32->p_dt
            nc.vector.memset(W, 0.0)
            nc.vector.memset(ct, 1.0)

            x_bmaj = x_in[:].rearrange("t b c -> b t c")  # [BLOC, T, C]

            for i in range(NT):
                t0 = i * TQ
                xt = xp.tile([128, C], f32, tag="xt")
                # partitions: p = 32*b + tau  <-  x[t0+tau, b, :]
                # One DMA per b so each transfer's outer dim is 32 (not 4):
                # descriptors then spread across the SDMA engine pool instead
                # of landing on 4 engines.
                for b in range(BLOC):
                    nc.gpsimd.dma_start(
                        out=xt[32 * b : 32 * (b + 1), :],
                        in_=x_bmaj[b, t0 : t0 + TQ, :],
                    )
                sums = sm.tile([128, 1], f32, tag="sums")
                scratch = scr.tile([128, C], p_dt, tag="scratch")
                nc.scalar.activation(
                    out=scratch,
                    in_=xt,
                    func=mybir.ActivationFunctionType.Exp,
                    accum_out=sums,
                )
                rec = sm.tile([128, 1], f32, tag="rec")
                nc.vector.reciprocal(out=rec, in_=sums)
                graw = sm.tile([128, NIDX], f32, tag="graw")
                nc.gpsimd.ap_gather(
                    out_ap=graw[:],
                    in_ap=xt[:],
                    idxs_ap=idx_t[:],
                    channels=128,
                    num_elems=C,
                    d=1,
                    num_idxs=NIDX,
                )
                e = sm.tile([128, NIDX], f32, tag="e")
                nc.scalar.activation(
                    out=e,
                    in_=graw,
                    func=mybir.ActivationFunctionType.Exp,
                    bias=lnk_t[:],
                )
                # p/q construction on GPSIMD: keeps the DVE instruction stream
                # free for the DP chain (avoids head-of-line stalls between
                # DP chunks and the next tile's pq production)
                pqs = sm.tile([128, PQW], p_dt, tag="pqs")
                nc.gpsimd.tensor_scalar_mul(
                    out=pqs[:, 0:NIDX], in0=e, scalar1=rec
                )
                nc.gpsimd.tensor_mul(
                    out=pqs[:, NIDX:PQW], in0=pqs[:, 0:NIDX], in1=msk_t
                )
                # assemble: pq_i[b, tau*128 + j] = pqs[32*b + tau, j]
                pq = pq_tiles[i]
                for b in range(BLOC):
                    nc.sync.dma_start(
                        out=pq[b : b + 1, :],
                        in_=pqs[32 * b : 32 * (b + 1), :],
                    )

                # ---- DP chunk for t in [t0, t0+TQ) ----
                for t in range(max(t0, 1), t0 + TQ):
                    if t == 1:
                        # init alpha_0 from p'_0 at s=0,1
                        nc.vector.tensor_copy(out=W[:, 2:4], in_=pq[:, 0:2])
                    # u[s] = alpha[s] + alpha[s-1]
                    nc.vector.scalar_tensor_tensor(
                        out=u,
                        in0=W[:, 2 : 2 + S],
                        scalar=0.0,
                        in1=W[:, 1 : 1 + S],
                        op0=mybir.AluOpType.bypass,
                        op1=mybir.AluOpType.add,
                    )
                    # z[s] = u[s] + B[s]  (B[s] = m[s]*alpha[s-2])
                    nc.vector.scalar_tensor_tensor(
                        out=z,
                        in0=u,
                        scalar=0.0,
                        in1=W[:, NIDX : NIDX + S],
                        op0=mybir.AluOpType.bypass,
                        op1=mybir.AluOpType.add,
                    )
                    if t % RENORM == 0:
                        nc.vector.reciprocal(out=r, in_=ct[:, t - 1 : t])
                        op0, sc = mybir.AluOpType.mult, r[:]
                    else:
                        op0, sc = mybir.AluOpType.bypass, 0.0
                    # fused update: {A[2:53], B[66:117]} = (z*r) * {p_t, q_t}
                    z_b = bass.AP(
                        tensor=z.tensor, offset=z.offset, ap=[z.ap[0], [0, 2], [1, S]]
                    )
                    w_o = bass.AP(
                        tensor=W.tensor,
                        offset=W.offset + 2,
                        ap=[W.ap[0], [NIDX, 2], [1, S]],
                    )
                    pq_t = bass.AP(
                        tensor=pq.tensor,
                        offset=pq.offset + (t - t0) * PQW,
                        ap=[pq.ap[0], [NIDX, 2], [1, S]],
                    )
                    nc.vector.scalar_tensor_tensor(
                        out=w_o,
                        in0=z_b,
                        scalar=sc,
                        in1=pq_t,
                        op0=op0,
                        op1=mybir.AluOpType.mult,
                        accum_out=ct[:, t : t + 1],
                    )

            nc.sync.dma_start(out=a_out[:], in_=W[:, 0 : S + 2])
            nc.sync.dma_start(out=ct_out[:], in_=ct)
    nc.finalize()
    return nc


def _prep_labels(label):
    pos = np.arange(L)
    key = np.where(label != 0, pos, L + pos)
    order = np.argsort(key, axis=1, kind="stable")
    packed = np.take_along_axis(label, order, axis=1)
    label_len = (label != 0).sum(1)
    ext = np.zeros((B, S), dtype=np.int64)
    ext[:, 1::2] = packed
    ext_m2 = np.pad(ext, ((0, 0), (2, 0)))[:, :S]
    skip = (np.arange(S) >= 2) & (ext != 0) & (ext != ext_m2)
    return ext, skip.astype(np.float32), label_len


def _core_inputs(x, ext, skip, core):
    bs = core * BLOC
    xc = np.ascontiguousarray(x[:, bs : bs + BLOC, :], dtype=np.float32)
    # gather index tile: 16-partition group g covers b = g//2; index j of the
    # group's list lives at [16*g + j%16, j//16]
    idx = np.zeros((128, NIDX // 16), dtype=np.int16)
    for g in range(8):
        b = bs + g // 2
        lst = np.zeros(NIDX, dtype=np.int16)
        lst[:S] = ext[b].astype(np.int16)
        idx[16 * g : 16 * (g + 1), :] = lst.reshape(NIDX // 16, 16).T
    # mask tile (pre-shifted): msk[32*b + tau, c] = skip[b, c+2]
    msk = np.zeros((128, NIDX), dtype=np.float32)
    for bl in range(BLOC):
        row = np.zeros(NIDX, dtype=np.float32)
        row[: S - 2] = skip[bs + bl, 2:]
        msk[32 * bl : 32 * (bl + 1), :] = row[None, :]
    return {"x": xc, "idx": idx, "msk": msk}


def _host_loss(a_fin, ct, label_len, core):
    """a_fin [BLOC, S+2] f32, ct [BLOC, T] f32 -> per-b losses (f64)."""
    losses = np.zeros(BLOC)
    renorm_ts = np.arange(RENORM, T, RENORM)
    for bl in range(BLOC):
        l = int(label_len[core * BLOC + bl])
        i0, i1 = max(2 * l - 1, 0), 2 * l
        end = float(a_fin[bl, 2 + i0]) + float(a_fin[bl, 2 + i1])
        cts = ct[bl, renorm_ts - 1].astype(np.float64)
        if end <= 0 or not np.isfinite(end) or np.any(~np.isfinite(cts)) or np.any(cts <= 0):
            losses[bl] = 0.0
            continue
        logP = np.log(end) + np.log(cts).sum() - T * LNK
        lb = -logP
        losses[bl] = 0.0 if lb > 1e8 else lb
    return losses


def kernel(input, label, p_dtype="bfloat16", state_dtype="float32", trace=False):
    global LAST_RESULT
    x = np.asarray(input, dtype=np.float32)
    label = np.asarray(label).astype(np.int64)
    assert x.shape == (T, B, C) and label.shape == (B, L)
    ext, skip, label_len = _prep_labels(label)

    key = (p_dtype, state_dtype)
    if key not in _CACHE:
        _CACHE[key] = _build_bass(
            getattr(mybir.dt, p_dtype), getattr(mybir.dt, state_dtype)
        )
    nc = _CACHE[key]

    in_maps = [_core_inputs(x, ext, skip, c) for c in range(NCORES)]
    res = run_bass_kernel_spmd(nc, in_maps, list(range(NCORES)), trace=trace)
    LAST_RESULT = res
    losses = np.concatenate(
        [
            _host_loss(
                np.asarray(res.results[c]["a_out"], np.float32),
                np.asarray(res.results[c]["ct_out"], np.float32),
                label_len,
                c,
            )
            for c in range(NCORES)
        ]
    )
    return np.float32(losses.mean() / T)
